# revision 6
# baseline (speedup 1.0000x reference)
"""Trainium2 Bass kernel for nn_AttentionProbe_80891414053184.

Math (reference):
    y  = relu(x @ W1.T + b1)            # (B,S,H) -> (B,S,128)
    y2 = relu(y @ W2.T + b2)            # (B,S,128)
    l  = y2 @ Wq.T + pos*pos_w  (+mask) # (B,S,8) logits
    p  = softmax(l, axis=S)
    v  = y2 @ Wv.T + bv
    out[b] = sum_{s,h} p*v + bias       # (B,1)

Strategy: sequence-parallel over 8 cores (512 positions x 4 batches = 2048
tokens per core).  Each core streams its x-shard, runs the MLP + head
projections on-chip, and emits per-(batch, head) partial softmax stats
(Z=sum exp(l-K), W'=sum exp(l-K)*v_raw) for a HOST-CHOSEN shift K (the
max of the ALiBi ramp + mask term over the shard, known without looking
at the data).  The host merges the 8 partial stats with the standard
online-softmax combine (m=K per core) and produces the (4,1) output.

Perf decisions (from HW traces of earlier versions):
 - x and W1 travel as fp8 e4m3 (W1 pre-scaled by 64 so its sigma~1/64
   values leave the fp8 denormal range; 1/64 is folded into W2, exact by
   relu's positive homogeneity).  Halves the dominant HBM stream vs bf16;
   measured end-to-end rel-err ~4e-3 (gate is 2e-2).
 - Layer-1 matmuls run perf_mode=DoubleRow: K=256 per instruction.
 - x streams via BOTH HWDGE rings (nc.sync + nc.scalar, alternating 1MB
   transfers): a single ring leaves a ~0.6us completion-receipt bubble
   between transfers (measured 74% duty); alternation hides it.
 - Constants ride the SWDGE (gpsimd) ring so they never block the x
   stream's HWDGE FIFOs.
 - The whole fp8 x-shard (64KB/partition) stays resident in SBUF.
 - The last super-chunk is DMA'd per token tile so tile t's MLP tail
   overlaps tile t+1's final transfer.
 - ~14 dummy matmuls on zeroed SBUF warm the PE HAM clock gate during the
   first DMA wait.
 - No on-device softmax max-reduce and no bv add: K is baked into the
   additive term `ca`, and W = W' + bv*Z happens in the host merge.
"""

import numpy as np

# Problem dims (hardcoded per harness contract).
B, S, H = 4, 4096, 4096
MLP, NH = 128, 8
NCORES = 8
S_SHARD = S // NCORES        # 512 seq positions per core
TOK = B * S_SHARD            # 2048 tokens per core
NT = TOK // 512              # 4 token tiles of 512 (= one batch each)
KCH = H // 128               # 32 contraction chunks of 128
NSUP = KCH // 2              # 16 DoubleRow super-chunks of 256
NBIG = (NSUP - 1) // 2       # 7 big 1MB x transfers (super-chunks 0..13)
P32 = NT * NH                # 32 packed (tile, head) lanes
W1_SCALE = 64.0              # lifts W1 (sigma 1/64) out of fp8 denormals

_cache = {}


def _build_nc():
    import concourse.mybir as mybir
    import concourse.tile as tile
    from concourse import bacc

    f32 = mybir.dt.float32
    f32r = mybir.dt.float32r
    fp8 = mybir.dt.float8e4
    DR = mybir.MatmulPerfMode.DoubleRow

    AF = mybir.ActivationFunctionType
    AX = mybir.AxisListType
    OP = mybir.AluOpType
    CQ = MLP                    # wq32 blocks start (cols of cw/cwr)
    CV = MLP + P32 * NT         # wv32 blocks start
    CB1 = MLP + 2 * P32 * NT    # col index of 64*b1 in cw; +1 is b2

    nc = bacc.Bacc()
    # x, packed on host: xt[p, k, n] = x^T[k*128+p, n]  (fp8)
    xt_d = nc.dram_tensor("xt", [128, NSUP, 2, TOK], fp8, kind="ExternalInput")
    # W1*64 packed likewise: w1s[p, k, m] = 64*W1[m, k*128+p]  (fp8)
    w1_d = nc.dram_tensor("w1s", [128, NSUP, 2, MLP], fp8, kind="ExternalInput")
    cwr_d = nc.dram_tensor("cwr", [MLP, MLP + 2 * P32 * NT], f32r,
                           kind="ExternalInput")
    cw_d = nc.dram_tensor("cw", [MLP, MLP + 2 * P32 * NT + 2], f32,
                          kind="ExternalInput")
    ca_d = nc.dram_tensor("ca", [P32, 512], f32, kind="ExternalInput")
    st_d = nc.dram_tensor("stats", [P32, 2], f32, kind="ExternalOutput")

    with tile.TileContext(nc) as tc:
        with (
            tc.tile_pool(name="const", bufs=1) as const,
            tc.tile_pool(name="xp", bufs=NBIG + NT) as xp,
            tc.tile_pool(name="yp", bufs=4) as yp,
            tc.tile_pool(name="y2p", bufs=4) as y2p,
            tc.tile_pool(name="smallp", bufs=1) as smallp,
            tc.tile_pool(name="statsp", bufs=1) as statsp,
            tc.tile_pool(name="ps_y", bufs=4, space="PSUM") as ps_y,
            tc.tile_pool(name="ps_y2", bufs=2, space="PSUM") as ps_y2,
            tc.tile_pool(name="ps_q", bufs=1, space="PSUM") as ps_q,
            tc.tile_pool(name="ps_v", bufs=1, space="PSUM") as ps_v,
        ):
            # --- HAM warmup: zeroed fp8 tiles + dummy matmuls keep the PE
            # busy through one full 4096-cycle activity window during the
            # first x DMA, so the clock gate opens before real work lands.
            warm_w = const.tile([128, 2, MLP], fp8)
            nc.gpsimd.memset(warm_w[:], 0.0)
            warm_x = const.tile([128, 2, 512], fp8)
            nc.gpsimd.memset(warm_x[:], 0.0)
            warm_ps = ps_y2.tile([128, 512], f32, tag="y2", name="warm_ps")
            warm_mm = None
            for i in range(14):
                warm_mm = nc.tensor.matmul(warm_ps[:], warm_w[:], warm_x[:],
                                           start=True, stop=True,
                                           perf_mode=DR)

            # --- Constants on the SWDGE ring (HWDGE rings belong to x).
            w1_sb = const.tile([128, NSUP, 2, MLP], fp8)
            nc.gpsimd.dma_start(out=w1_sb[:], in_=w1_d[:])
            ca_sb = const.tile([P32, 512], f32)
            nc.gpsimd.dma_start(out=ca_sb[:], in_=ca_d[:])
            cw_sb = const.tile([MLP, MLP + 2 * P32 * NT + 2], f32)
            nc.gpsimd.dma_start(out=cw_sb[:], in_=cw_d[:])
            cwr_sb = const.tile([MLP, MLP + 2 * P32 * NT], f32r)
            nc.gpsimd.dma_start(out=cwr_sb[:], in_=cwr_d[:])

            stats_sb = statsp.tile([P32, 2], f32)

            # --- x stream alternating across both HWDGE rings: 7 x 1MB
            # (super-chunk pairs 0..13), then the last super-chunk split
            # per token tile (4 x 256KB).
            rings = [nc.sync, nc.scalar]
            x_tiles = []
            for j in range(NBIG):
                x_sb = xp.tile([128, 2, 2, TOK], fp8, tag="x", name=f"x{j}")
                rings[j % 2].dma_start(
                    out=x_sb[:], in_=xt_d[:, 2 * j:2 * j + 2])
                x_tiles.append(x_sb)
            xl_tiles = []
            for t in range(NT):
                xl_sb = xp.tile([128, 2, 2, 512], fp8, tag="xl", name=f"xl{t}")
                rings[(NBIG + t) % 2].dma_start(
                    out=xl_sb[:],
                    in_=xt_d[:, NSUP - 2:NSUP, :, t * 512:(t + 1) * 512])
                xl_tiles.append(xl_sb)

            # --- Layer 1: yT[t] (128, 512) += (64*W1)^T-chunk @ x-chunk,
            # DoubleRow accumulation over 16 super-chunks of K=256.
            psum_y = [ps_y.tile([128, 512], f32, tag="y", name=f"y_ps{t}")
                      for t in range(NT)]
            for g in range(2 * NBIG):
                xg = x_tiles[g // 2][:, g % 2]
                for t in range(NT):
                    mm = nc.tensor.matmul(
                        psum_y[t][:],
                        w1_sb[:, g],
                        xg[:, :, t * 512:(t + 1) * 512],
                        start=(g == 0), stop=False,
                        perf_mode=DR,
                    )
                    if g == 0 and t == 0 and warm_mm is not None:
                        tile.add_dep_helper(mm.ins, warm_mm.ins, sync=False,
                                            reason="warmups before first mm")
            for t in range(NT):
                nc.tensor.matmul(psum_y[t][:], w1_sb[:, NSUP - 2],
                                 xl_tiles[t][:, 0],
                                 start=False, stop=False, perf_mode=DR)
                nc.tensor.matmul(psum_y[t][:], w1_sb[:, NSUP - 1],
                                 xl_tiles[t][:, 1],
                                 start=False, stop=True, perf_mode=DR)

            # cw/ca lane warmups (each engine observes the const lanes once
            # so steady-state instructions carry at most one new wait).
            warm_act = const.tile([MLP, 1], f32)
            nc.scalar.copy(out=warm_act[:], in_=cw_sb[:, CB1:CB1 + 1])
            warm_dve = const.tile([P32, 1], f32)
            nc.vector.tensor_copy(out=warm_dve[:], in_=ca_sb[:, 0:1])
            warm_pe2 = ps_y2.tile([128, 512], f32, tag="y2", name="warm_pe2")
            nc.tensor.matmul(warm_pe2[0:NH, 0:NH], cwr_sb[:, 0:NH],
                             cwr_sb[:, 0:NH], start=True, stop=True)

            # --- MLP tail + head projections per token tile.
            q32_ps = ps_q.tile([P32, 512], f32, tag="q", name="q32_ps")
            v32_ps = ps_v.tile([P32, 512], f32, tag="v", name="v32_ps")
            for t in range(NT):
                y_sb = yp.tile([128, 512], f32r, tag="ysb", name=f"y_sb{t}")
                # relu on DVE (add+max) keeps ACT free for relu2/exp; the
                # 64x scale rides along and is cancelled by W2/64 in cwr.
                nc.vector.tensor_scalar(out=y_sb[:], in0=psum_y[t][:],
                                        scalar1=cw_sb[:, CB1:CB1 + 1],
                                        scalar2=0.0, op0=OP.add, op1=OP.max)
                y2_ps = ps_y2.tile([128, 512], f32, tag="y2", name=f"y2_ps{t}")
                nc.tensor.matmul(y2_ps[:], cwr_sb[:, 0:MLP], y_sb[:],
                                 start=True, stop=True)
                y2_sb = y2p.tile([128, 512], f32r, tag="y2sb", name=f"y2_sb{t}")
                nc.scalar.activation(out=y2_sb[:], in_=y2_ps[:], func=AF.Relu,
                                     bias=cw_sb[:, CB1 + 1:CB1 + 2], scale=1.0)
                # Head projections: the (128, 32) weight block for tile t is
                # zero outside rows 8t..8t+8, so accumulating all 4 tiles into
                # one (32, 512) bank packs q/v as (tile, head) x seq lanes.
                nc.tensor.matmul(q32_ps[:],
                                 cwr_sb[:, CQ + P32 * t:CQ + P32 * (t + 1)],
                                 y2_sb[:], start=(t == 0), stop=(t == NT - 1))
                nc.tensor.matmul(v32_ps[:],
                                 cwr_sb[:, CV + P32 * t:CV + P32 * (t + 1)],
                                 y2_sb[:], start=(t == 0), stop=(t == NT - 1))

            # --- Softmax stats over the packed (32, 512) lanes.
            # ca already contains ramp + mask - K, so l' = q + ca is the
            # shifted logit; no max-reduce needed on device.
            l_sb = smallp.tile([P32, 512], f32, tag="l", name="l_sb")
            nc.vector.tensor_add(out=l_sb[:], in0=q32_ps[:],
                                 in1=ca_sb[:])
            e_sb = smallp.tile([P32, 512], f32, tag="e", name="e_sb")
            # e = exp(l'); stats[:, 0] = Z = sum e
            nc.scalar.activation(out=e_sb[:], in_=l_sb[:], func=AF.Exp,
                                 bias=0.0, scale=1.0,
                                 accum_out=stats_sb[:, 0:1])
            ev_sb = smallp.tile([P32, 512], f32, tag="ev", name="ev_sb")
            nc.vector.tensor_mul(out=ev_sb[:], in0=e_sb[:], in1=v32_ps[:])
            # stats[:, 1] = W' = sum e*v_raw   (bv folded in on host)
            nc.vector.tensor_reduce(out=stats_sb[:, 1:2], in_=ev_sb[:],
                                    axis=AX.X, op=OP.add)

            nc.sync.dma_start(out=st_d[:], in_=stats_sb[:])

    nc.finalize()
    return nc


def get_nc():
    if "nc" not in _cache:
        _cache["nc"] = _build_nc()
    return _cache["nc"]


def make_core_inputs(x, mask, W1, b1, W2, b2, Wq, Wv, bv, pos_w, bias):
    """Host-side shard + transpose + fp8 quantization.

    Returns (in_maps, K) where K[c, t, h] is the logit shift baked into
    core c's `ca` (the host-known max of ramp+mask over the shard)."""
    import ml_dtypes
    fp8 = ml_dtypes.float8_e4m3

    # w1s[p, k, m] = 64*W1[m, k*128+p], fp8
    w1s = np.ascontiguousarray(
        (W1 * W1_SCALE).reshape(MLP, KCH, 128).transpose(2, 1, 0)
    ).astype(fp8).reshape(128, NSUP, 2, MLP)

    cw = np.zeros((MLP, MLP + 2 * P32 * NT + 2), dtype=np.float32)
    cw[:, 0:MLP] = W2.T / W1_SCALE
    # zero-padded per-tile head blocks: block t covers psum rows 8t..8t+8
    for t in range(NT):
        cw[:, MLP + P32 * t + NH * t:MLP + P32 * t + NH * (t + 1)] = Wq.T
        base_v = MLP + P32 * NT
        cw[:, base_v + P32 * t + NH * t:base_v + P32 * t + NH * (t + 1)] = Wv.T
    cw[:, MLP + 2 * P32 * NT] = b1 * W1_SCALE
    cw[:, MLP + 2 * P32 * NT + 1] = b2
    cwr = np.ascontiguousarray(cw[:, 0:MLP + 2 * P32 * NT])
    pos = np.arange(S, dtype=np.float32)
    maskadd = np.where(mask == 0, np.float32(-1e9), np.float32(0.0))  # (B,S)

    in_maps = []
    K_all = np.empty((NCORES, NT, NH), dtype=np.float64)
    for c in range(NCORES):
        sl = slice(c * S_SHARD, (c + 1) * S_SHARD)
        # xt[p, k, n] = x^T[k*128+p, n] for this shard's 2048 tokens
        xt = np.ascontiguousarray(
            x[:, sl, :].astype(fp8).transpose(2, 0, 1).reshape(H, TOK)
            .reshape(KCH, 128, TOK).transpose(1, 0, 2)
        ).reshape(128, NSUP, 2, TOK)
        add_ths = (pos_w.astype(np.float64)[None, :, None]
                   * pos[sl].astype(np.float64)[None, None, :]
                   + maskadd[:, None, sl])           # (B=NT, NH, 512)
        # K: host-known shift; floor keeps exp() sane when fully masked
        K = np.maximum(add_ths.max(axis=2), -60.0)   # (NT, NH)
        K_all[c] = K
        ca = (add_ths - K[:, :, None]).astype(np.float32).reshape(P32, 512)
        in_maps.append({"xt": xt, "w1s": w1s, "cw": cw, "cwr": cwr,
                        "ca": ca})
    return in_maps, K_all


def merge_stats(stats_all, K_all, bv, bias):
    """stats_all: (NCORES, 32, 2) = [Z, W'] per (batch tile, head) lane
    under shift K_all[c, t, h]; W = W' + bv*Z -> (B, 1) output."""
    st = np.asarray(stats_all, dtype=np.float64).reshape(NCORES, NT, NH, 2)
    m = K_all                # (C, B, NH): logits were shifted by -K
    Z = st[..., 0]
    W = st[..., 1] + bv.astype(np.float64)[None, None, :] * Z
    M = m.max(axis=0)        # (B, NH)
    alpha = np.exp(m - M[None])
    Zg = (alpha * Z).sum(axis=0)
    Wg = (alpha * W).sum(axis=0)
    out = (Wg / Zg).sum(axis=1)          # (B,)
    return (out[:, None] + np.float64(bias.reshape(1)[0])).astype(np.float32)


def kernel(x, mask, W1, b1, W2, b2, Wq, Wv, bv, pos_w, bias, _trace=False):
    from concourse.bass_utils import run_bass_kernel_spmd

    x = np.asarray(x, dtype=np.float32)
    args = [np.asarray(a) for a in (W1, b1, W2, b2, Wq, Wv, bv, pos_w, bias)]
    in_maps, K_all = make_core_inputs(x, np.asarray(mask), *args)
    nc = get_nc()
    res = run_bass_kernel_spmd(nc, in_maps, core_ids=list(range(NCORES)),
                               trace=_trace)
    stats_all = np.stack([r["stats"] for r in res.results])  # (C, 32, 2)
    out = merge_stats(stats_all, K_all, args[6], args[8])
    if _trace:
        kernel.last_result = res
    return out


# revision 7
# speedup vs baseline: 1.1980x; 1.1980x over previous
"""Trainium2 Bass kernel for nn_AttentionProbe_80891414053184.

Math (reference):
    y  = relu(x @ W1.T + b1)            # (B,S,H) -> (B,S,128)
    y2 = relu(y @ W2.T + b2)            # (B,S,128)
    l  = y2 @ Wq.T + pos*pos_w  (+mask) # (B,S,8) logits
    p  = softmax(l, axis=S)
    v  = y2 @ Wv.T + bv
    out[b] = sum_{s,h} p*v + bias       # (B,1)

Strategy: sequence-parallel over 8 cores (512 positions x 4 batches = 2048
tokens per core).  Each core streams its x-shard, runs the MLP + head
projections on-chip, and emits per-(batch, head) partial softmax stats
(Z=sum exp(l-K), W'=sum exp(l-K)*v_raw) for a HOST-CHOSEN shift K (the
max of the ALiBi ramp + mask term over the shard, known without looking
at the data).  The host merges the 8 partial stats with the standard
online-softmax combine (m=K per core) and produces the (4,1) output.

Perf decisions (from HW traces of earlier versions):
 - x and W1 travel as fp8 e4m3 (W1 pre-scaled by 64 so its sigma~1/64
   values leave the fp8 denormal range; 1/64 is folded into W2, exact by
   relu's positive homogeneity).  Halves the dominant HBM stream vs bf16.
 - Layer-1 matmuls run perf_mode=DoubleRow: K=256 per instruction.
 - x streams via BOTH HWDGE rings (nc.sync + nc.scalar) in 1MB transfers,
   soft-dep-chained so per-ring delivery order == PE consumption order
   (the tile scheduler otherwise reorders and starves the PE mid-stream).
 - W2/Wq/Wv and the MLP tail activations run in bf16 (halves the const
   stream and doubles DVE throughput); measured end-to-end rel-err ~6e-3
   against a 2e-2 gate.
 - The whole fp8 x-shard (64KB/partition) stays resident in SBUF.
 - The last super-chunk pair is DMA'd per token tile so tile t's MLP tail
   overlaps tile t+1's final transfer.
 - ~14 dummy matmuls on zeroed SBUF warm the PE HAM clock gate during the
   first DMA wait.
 - No on-device softmax max-reduce and no bv add: K is baked into the
   additive term `ca`, and W = W' + bv*Z happens in the host merge.
"""

import numpy as np

# Problem dims (hardcoded per harness contract).
B, S, H = 4, 4096, 4096
MLP, NH = 128, 8
NCORES = 8
S_SHARD = S // NCORES        # 512 seq positions per core
TOK = B * S_SHARD            # 2048 tokens per core
NT = TOK // 512              # 4 token tiles of 512 (= one batch each)
KCH = H // 128               # 32 contraction chunks of 128
NSUP = KCH // 2              # 16 DoubleRow super-chunks of 256
NBIG = (NSUP - 2) // 2       # 7 big 1MB x transfers (super-chunks 0..13)
P32 = NT * NH                # 32 packed (tile, head) lanes
W1_SCALE = 64.0              # lifts W1 (sigma 1/64) out of fp8 denormals

_cache = {}


def _build_nc():
    import concourse.mybir as mybir
    import concourse.tile as tile
    from concourse import bacc
    from concourse.tile import add_dep_helper

    f32 = mybir.dt.float32
    bf16 = mybir.dt.bfloat16
    fp8 = mybir.dt.float8e4
    DR = mybir.MatmulPerfMode.DoubleRow

    AF = mybir.ActivationFunctionType
    AX = mybir.AxisListType
    OP = mybir.AluOpType
    CQ = MLP                    # wq32 blocks start (cols of cwr)
    CV = MLP + P32 * NT         # wv32 blocks start

    nc = bacc.Bacc()
    # x, packed on host: xt[p, k, n] = x^T[k*128+p, n]  (fp8)
    xt_d = nc.dram_tensor("xt", [128, NSUP, 2, TOK], fp8, kind="ExternalInput")
    # W1*64 packed likewise: w1s[p, k, m] = 64*W1[m, k*128+p]  (fp8)
    w1_d = nc.dram_tensor("w1s", [128, NSUP, 2, MLP], fp8, kind="ExternalInput")
    # cwr: [W2.T/64 | wq32 (4 x 32-wide zero-padded blocks) | wv32]  (bf16)
    cwr_d = nc.dram_tensor("cwr", [MLP, MLP + 2 * P32 * NT], bf16,
                           kind="ExternalInput")
    # cb: bias columns [64*b1 | b2]  (f32)
    cb_d = nc.dram_tensor("cb", [MLP, 2], f32, kind="ExternalInput")
    ca_d = nc.dram_tensor("ca", [P32, 512], f32, kind="ExternalInput")
    st_d = nc.dram_tensor("stats", [P32, 2], f32, kind="ExternalOutput")

    with tile.TileContext(nc) as tc:
        with (
            tc.tile_pool(name="const", bufs=1) as const,
            tc.tile_pool(name="xp", bufs=NBIG + NT) as xp,
            tc.tile_pool(name="yp", bufs=4) as yp,
            tc.tile_pool(name="y2p", bufs=4) as y2p,
            tc.tile_pool(name="smallp", bufs=1) as smallp,
            tc.tile_pool(name="statsp", bufs=1) as statsp,
            tc.tile_pool(name="ps_y", bufs=4, space="PSUM") as ps_y,
            tc.tile_pool(name="ps_y2", bufs=2, space="PSUM") as ps_y2,
            tc.tile_pool(name="ps_q", bufs=1, space="PSUM") as ps_q,
            tc.tile_pool(name="ps_v", bufs=1, space="PSUM") as ps_v,
        ):
            # --- HAM warmup: zeroed fp8 tiles + dummy matmuls keep the PE
            # busy through one full 4096-cycle activity window during the
            # first x DMA, so the clock gate opens before real work lands.
            warm_w = const.tile([128, 2, MLP], fp8)
            nc.gpsimd.memset(warm_w[:], 0.0)
            warm_x = const.tile([128, 2, 512], fp8)
            nc.gpsimd.memset(warm_x[:], 0.0)
            warm_ps = ps_y2.tile([128, 512], f32, tag="y2", name="warm_ps")
            warm_mm = None
            for i in range(14):
                warm_mm = nc.tensor.matmul(warm_ps[:], warm_w[:], warm_x[:],
                                           start=True, stop=True,
                                           perf_mode=DR)

            # --- DMA plan.  Ring A (sync): x0,x2,x4,x6,xl0,xl2,stats.
            # Ring B (scalar): w1,x1,x3,x5,xl1,xl3.  Small consts on the
            # SWDGE ring.  Soft deps pin per-ring issue order; the lane
            # sems then deliver in consumption order.
            last_on = {}

            def ring_dma(ring_key, ring, **kw):
                dma = ring.dma_start(**kw)
                prev = last_on.get(ring_key)
                if prev is not None:
                    add_dep_helper(dma.ins, prev.ins, sync=False,
                                   reason="ring issue order")
                last_on[ring_key] = dma
                return dma

            w1_sb = const.tile([128, NSUP, 2, MLP], fp8)
            ring_dma('B', nc.scalar, out=w1_sb[:], in_=w1_d[:])
            ca_sb = const.tile([P32, 512], f32)
            nc.gpsimd.dma_start(out=ca_sb[:], in_=ca_d[:])
            cb_sb = const.tile([MLP, 2], f32)
            nc.gpsimd.dma_start(out=cb_sb[:], in_=cb_d[:])
            cwr_sb = const.tile([MLP, MLP + 2 * P32 * NT], bf16)
            nc.gpsimd.dma_start(out=cwr_sb[:], in_=cwr_d[:])

            stats_sb = statsp.tile([P32, 2], f32)

            rings = [('A', nc.sync), ('B', nc.scalar)]
            x_tiles = []
            for j in range(NBIG):
                x_sb = xp.tile([128, 2, 2, TOK], fp8, tag="x", name=f"x{j}")
                rk, ring = rings[j % 2]
                ring_dma(rk, ring, out=x_sb[:], in_=xt_d[:, 2 * j:2 * j + 2])
                x_tiles.append(x_sb)
            xl_tiles = []
            for t in range(NT):
                xl_sb = xp.tile([128, 2, 2, 512], fp8, tag="xl", name=f"xl{t}")
                rk, ring = rings[(NBIG + t) % 2]
                ring_dma(rk, ring, out=xl_sb[:],
                         in_=xt_d[:, NSUP - 2:NSUP, :, t * 512:(t + 1) * 512])
                xl_tiles.append(xl_sb)

            # --- Layer 1: yT[t] (128, 512) += (64*W1)^T-chunk @ x-chunk,
            # DoubleRow accumulation over 16 super-chunks of K=256.
            psum_y = [ps_y.tile([128, 512], f32, tag="y", name=f"y_ps{t}")
                      for t in range(NT)]
            for g in range(2 * NBIG):
                xg = x_tiles[g // 2][:, g % 2]
                for t in range(NT):
                    mm = nc.tensor.matmul(
                        psum_y[t][:],
                        w1_sb[:, g],
                        xg[:, :, t * 512:(t + 1) * 512],
                        start=(g == 0), stop=False,
                        perf_mode=DR,
                    )
                    if g == 0 and t == 0 and warm_mm is not None:
                        add_dep_helper(mm.ins, warm_mm.ins, sync=False,
                                       reason="warmups before first mm")
            for t in range(NT):
                nc.tensor.matmul(psum_y[t][:], w1_sb[:, NSUP - 2],
                                 xl_tiles[t][:, 0],
                                 start=False, stop=False, perf_mode=DR)
                nc.tensor.matmul(psum_y[t][:], w1_sb[:, NSUP - 1],
                                 xl_tiles[t][:, 1],
                                 start=False, stop=True, perf_mode=DR)

            # cb/ca lane warmups (each engine observes the const lanes once
            # so steady-state instructions carry at most one new wait).
            warm_act = const.tile([MLP, 1], f32)
            nc.scalar.copy(out=warm_act[:], in_=cb_sb[:, 0:1])
            warm_dve = const.tile([P32, 1], f32)
            nc.vector.tensor_copy(out=warm_dve[:], in_=ca_sb[:, 0:1])
            warm_pe2 = ps_y2.tile([128, 512], f32, tag="y2", name="warm_pe2")
            nc.tensor.matmul(warm_pe2[0:NH, 0:NH], cwr_sb[:, 0:NH],
                             cwr_sb[:, 0:NH], start=True, stop=True)

            # --- MLP tail + head projections per token tile (bf16).
            q32_ps = ps_q.tile([P32, 512], f32, tag="q", name="q32_ps")
            v32_ps = ps_v.tile([P32, 512], f32, tag="v", name="v32_ps")
            for t in range(NT):
                y_sb = yp.tile([128, 512], bf16, tag="ysb", name=f"y_sb{t}")
                # relu on DVE (add+max) keeps ACT free for relu2/exp; the
                # 64x scale rides along and is cancelled by W2/64 in cwr.
                nc.vector.tensor_scalar(out=y_sb[:], in0=psum_y[t][:],
                                        scalar1=cb_sb[:, 0:1],
                                        scalar2=0.0, op0=OP.add, op1=OP.max)
                y2_ps = ps_y2.tile([128, 512], f32, tag="y2", name=f"y2_ps{t}")
                nc.tensor.matmul(y2_ps[:], cwr_sb[:, 0:MLP], y_sb[:],
                                 start=True, stop=True)
                y2_sb = y2p.tile([128, 512], bf16, tag="y2sb", name=f"y2_sb{t}")
                nc.scalar.activation(out=y2_sb[:], in_=y2_ps[:], func=AF.Relu,
                                     bias=cb_sb[:, 1:2], scale=1.0)
                # Head projections: the (128, 32) weight block for tile t is
                # zero outside rows 8t..8t+8, so accumulating all 4 tiles into
                # one (32, 512) bank packs q/v as (tile, head) x seq lanes.
                nc.tensor.matmul(q32_ps[:],
                                 cwr_sb[:, CQ + P32 * t:CQ + P32 * (t + 1)],
                                 y2_sb[:], start=(t == 0), stop=(t == NT - 1))
                nc.tensor.matmul(v32_ps[:],
                                 cwr_sb[:, CV + P32 * t:CV + P32 * (t + 1)],
                                 y2_sb[:], start=(t == 0), stop=(t == NT - 1))

            # --- Softmax stats over the packed (32, 512) lanes.
            # ca already contains ramp + mask - K, so l' = q + ca is the
            # shifted logit; no max-reduce needed on device.
            l_sb = smallp.tile([P32, 512], f32, tag="l", name="l_sb")
            nc.vector.tensor_add(out=l_sb[:], in0=q32_ps[:],
                                 in1=ca_sb[:])
            e_sb = smallp.tile([P32, 512], f32, tag="e", name="e_sb")
            # e = exp(l'); stats[:, 0] = Z = sum e
            nc.scalar.activation(out=e_sb[:], in_=l_sb[:], func=AF.Exp,
                                 bias=0.0, scale=1.0,
                                 accum_out=stats_sb[:, 0:1])
            ev_sb = smallp.tile([P32, 512], f32, tag="ev", name="ev_sb")
            nc.vector.tensor_mul(out=ev_sb[:], in0=e_sb[:], in1=v32_ps[:])
            # stats[:, 1] = W' = sum e*v_raw   (bv folded in on host)
            nc.vector.tensor_reduce(out=stats_sb[:, 1:2], in_=ev_sb[:],
                                    axis=AX.X, op=OP.add)

            ring_dma('A', nc.sync, out=st_d[:], in_=stats_sb[:])

    nc.finalize()
    return nc


def get_nc():
    if "nc" not in _cache:
        _cache["nc"] = _build_nc()
    return _cache["nc"]


def make_core_inputs(x, mask, W1, b1, W2, b2, Wq, Wv, bv, pos_w, bias):
    """Host-side shard + transpose + fp8 quantization.

    Returns (in_maps, K) where K[c, t, h] is the logit shift baked into
    core c's `ca` (the host-known max of ramp+mask over the shard)."""
    import ml_dtypes
    fp8 = ml_dtypes.float8_e4m3
    bf16 = ml_dtypes.bfloat16

    # w1s[p, k, m] = 64*W1[m, k*128+p], fp8
    w1s = np.ascontiguousarray(
        (W1 * W1_SCALE).reshape(MLP, KCH, 128).transpose(2, 1, 0)
    ).astype(fp8).reshape(128, NSUP, 2, MLP)

    cwr = np.zeros((MLP, MLP + 2 * P32 * NT), dtype=np.float32)
    cwr[:, 0:MLP] = W2.T / W1_SCALE
    # zero-padded per-tile head blocks: block t covers psum rows 8t..8t+8
    for t in range(NT):
        cwr[:, MLP + P32 * t + NH * t:MLP + P32 * t + NH * (t + 1)] = Wq.T
        base_v = MLP + P32 * NT
        cwr[:, base_v + P32 * t + NH * t:base_v + P32 * t + NH * (t + 1)] = \
            Wv.T
    cwr = cwr.astype(bf16)
    cb = np.stack([b1 * W1_SCALE, b2], axis=1).astype(np.float32)  # (MLP, 2)
    pos = np.arange(S, dtype=np.float32)
    maskadd = np.where(mask == 0, np.float32(-1e9), np.float32(0.0))  # (B,S)

    in_maps = []
    K_all = np.empty((NCORES, NT, NH), dtype=np.float64)
    for c in range(NCORES):
        sl = slice(c * S_SHARD, (c + 1) * S_SHARD)
        # xt[p, k, n] = x^T[k*128+p, n] for this shard's 2048 tokens
        xt = np.ascontiguousarray(
            x[:, sl, :].astype(fp8).transpose(2, 0, 1).reshape(H, TOK)
            .reshape(KCH, 128, TOK).transpose(1, 0, 2)
        ).reshape(128, NSUP, 2, TOK)
        add_ths = (pos_w.astype(np.float64)[None, :, None]
                   * pos[sl].astype(np.float64)[None, None, :]
                   + maskadd[:, None, sl])           # (B=NT, NH, 512)
        # K: host-known shift; floor keeps exp() sane when fully masked
        K = np.maximum(add_ths.max(axis=2), -60.0)   # (NT, NH)
        K_all[c] = K
        ca = (add_ths - K[:, :, None]).astype(np.float32).reshape(P32, 512)
        in_maps.append({"xt": xt, "w1s": w1s, "cwr": cwr, "cb": cb,
                        "ca": ca})
    return in_maps, K_all


def merge_stats(stats_all, K_all, bv, bias):
    """stats_all: (NCORES, 32, 2) = [Z, W'] per (batch tile, head) lane
    under shift K_all[c, t, h]; W = W' + bv*Z -> (B, 1) output."""
    st = np.asarray(stats_all, dtype=np.float64).reshape(NCORES, NT, NH, 2)
    m = K_all                # (C, B, NH): logits were shifted by -K
    Z = st[..., 0]
    W = st[..., 1] + bv.astype(np.float64)[None, None, :] * Z
    M = m.max(axis=0)        # (B, NH)
    alpha = np.exp(m - M[None])
    Zg = (alpha * Z).sum(axis=0)
    Wg = (alpha * W).sum(axis=0)
    out = (Wg / Zg).sum(axis=1)          # (B,)
    return (out[:, None] + np.float64(bias.reshape(1)[0])).astype(np.float32)


def kernel(x, mask, W1, b1, W2, b2, Wq, Wv, bv, pos_w, bias, _trace=False):
    from concourse.bass_utils import run_bass_kernel_spmd

    x = np.asarray(x, dtype=np.float32)
    args = [np.asarray(a) for a in (W1, b1, W2, b2, Wq, Wv, bv, pos_w, bias)]
    in_maps, K_all = make_core_inputs(x, np.asarray(mask), *args)
    nc = get_nc()
    res = run_bass_kernel_spmd(nc, in_maps, core_ids=list(range(NCORES)),
                               trace=_trace)
    stats_all = np.stack([r["stats"] for r in res.results])  # (C, 32, 2)
    out = merge_stats(stats_all, K_all, args[6], args[8])
    if _trace:
        kernel.last_result = res
    return out


# revision 12
# speedup vs baseline: 1.3006x; 1.0856x over previous
"""Trainium2 Bass kernel for nn_AttentionProbe_80891414053184.

Math (reference):
    y  = relu(x @ W1.T + b1)            # (B,S,H) -> (B,S,128)
    y2 = relu(y @ W2.T + b2)            # (B,S,128)
    l  = y2 @ Wq.T + pos*pos_w  (+mask) # (B,S,8) logits
    p  = softmax(l, axis=S)
    v  = y2 @ Wv.T + bv
    out[b] = sum_{s,h} p*v + bias       # (B,1)

Strategy: sequence-parallel over 8 cores (512 positions x 4 batches = 2048
tokens per core).  Each core streams its x-shard, runs the MLP + head
projections on-chip, and emits per-(batch, head) partial softmax stats
(Z=sum exp(l-K), W'=sum exp(l-K)*v_raw) for a HOST-CHOSEN shift K (the
max of the ALiBi ramp + mask term over the shard, known without looking
at the data).  The host merges the 8 partial stats with the standard
online-softmax combine (m=K per core) and produces the (4,1) output.

Perf decisions (from HW traces of earlier versions):
 - x and W1 travel as fp8 e4m3 (W1 pre-scaled by 64 so its sigma~1/64
   values leave the fp8 denormal range; 1/64 is folded into W2, exact by
   relu's positive homogeneity).  Halves the dominant HBM stream vs bf16.
 - Layer-1 matmuls run perf_mode=DoubleRow: K=256 per instruction.
 - x streams via BOTH HWDGE rings (nc.sync + nc.scalar) in 1MB transfers,
   soft-dep-chained so per-ring delivery order == PE consumption order
   (the tile scheduler otherwise reorders and starves the PE mid-stream).
 - W2/Wq/Wv and the MLP tail activations run in bf16 (halves the const
   stream and doubles DVE throughput); measured end-to-end rel-err ~6e-3
   against a 2e-2 gate.
 - The whole fp8 x-shard (64KB/partition) stays resident in SBUF.
 - The last super-chunk pair is DMA'd per token tile so tile t's MLP tail
   overlaps tile t+1's final transfer.
 - ~14 dummy matmuls on zeroed SBUF warm the PE HAM clock gate during the
   first DMA wait.
 - No on-device softmax max-reduce and no bv add: K is baked into the
   additive term `ca`, and W = W' + bv*Z happens in the host merge.
"""

import numpy as np

# Problem dims (hardcoded per harness contract).
B, S, H = 4, 4096, 4096
MLP, NH = 128, 8
NCORES = 8
S_SHARD = S // NCORES        # 512 seq positions per core
TOK = B * S_SHARD            # 2048 tokens per core
NT = TOK // 512              # 4 token tiles of 512 (= one batch each)
KCH = H // 128               # 32 contraction chunks of 128
NSUP = KCH // 2              # 16 DoubleRow super-chunks of 256
NBIG = (NSUP - 2) // 2       # 7 big 1MB x transfers (super-chunks 0..13)
P32 = NT * NH                # 32 packed (tile, head) lanes
W1_SCALE = 64.0              # lifts W1 (sigma 1/64) out of fp8 denormals

_cache = {}


def _build_nc():
    import concourse.mybir as mybir
    import concourse.tile as tile
    from concourse import bacc
    from concourse.tile import add_dep_helper

    f32 = mybir.dt.float32
    bf16 = mybir.dt.bfloat16
    fp8 = mybir.dt.float8e4
    DR = mybir.MatmulPerfMode.DoubleRow

    AF = mybir.ActivationFunctionType
    AX = mybir.AxisListType
    OP = mybir.AluOpType
    CQ = MLP                    # wq32 blocks start (cols of cwr)
    CV = MLP + P32 * NT         # wv32 blocks start

    nc = bacc.Bacc()
    # x, packed on host: xt[p, k, n] = x^T[k*128+p, n]  (fp8)
    xt_d = nc.dram_tensor("xt", [128, NSUP, 2, TOK], fp8, kind="ExternalInput")
    # W1*64 packed likewise: w1s[p, k, m] = 64*W1[m, k*128+p]  (fp8)
    w1_d = nc.dram_tensor("w1s", [128, NSUP, 2, MLP], fp8, kind="ExternalInput")
    # cwr: [W2.T/64 | wq32 (4 x 32-wide zero-padded blocks) | wv32]  (bf16)
    cwr_d = nc.dram_tensor("cwr", [MLP, MLP + 2 * P32 * NT], bf16,
                           kind="ExternalInput")
    # cb: bias columns [64*b1 | b2]  (f32)
    cb_d = nc.dram_tensor("cb", [MLP, 2], f32, kind="ExternalInput")
    ca_d = nc.dram_tensor("ca", [P32, 512], f32, kind="ExternalInput")
    st_d = nc.dram_tensor("stats", [P32, 2], f32, kind="ExternalOutput")

    with tile.TileContext(nc) as tc:
        with (
            tc.tile_pool(name="const", bufs=1) as const,
            tc.tile_pool(name="xp", bufs=(NSUP - 2) + NT) as xp,
            tc.tile_pool(name="yp", bufs=4) as yp,
            tc.tile_pool(name="y2p", bufs=4) as y2p,
            tc.tile_pool(name="smallp", bufs=1) as smallp,
            tc.tile_pool(name="statsp", bufs=1) as statsp,
            tc.tile_pool(name="ps_y", bufs=4, space="PSUM") as ps_y,
            tc.tile_pool(name="ps_y2", bufs=2, space="PSUM") as ps_y2,
            tc.tile_pool(name="ps_q", bufs=1, space="PSUM") as ps_q,
            tc.tile_pool(name="ps_v", bufs=1, space="PSUM") as ps_v,
        ):
            # --- HAM warmup: zeroed fp8 tiles + dummy matmuls keep the PE
            # busy through one full 4096-cycle activity window during the
            # first x DMA, so the clock gate opens before real work lands.
            warm_w = const.tile([128, 2, MLP], fp8)
            nc.gpsimd.memset(warm_w[:], 0.0)
            warm_x = const.tile([128, 2, 512], fp8)
            nc.gpsimd.memset(warm_x[:], 0.0)
            warm_ps = ps_y2.tile([128, 512], f32, tag="y2", name="warm_ps")
            warm_mm = None
            for i in range(8):
                warm_mm = nc.tensor.matmul(warm_ps[:], warm_w[:], warm_x[:],
                                           start=True, stop=True,
                                           perf_mode=DR)

            # --- DMA plan.  Ring A (sync): x0,x2,x4,x6,xl0,xl2,stats.
            # Ring B (scalar): w1,x1,x3,x5,xl1,xl3.  Small consts on the
            # SWDGE ring.  Soft deps pin per-ring issue order; the lane
            # sems then deliver in consumption order.
            last_on = {}

            def ring_dma(ring_key, ring, **kw):
                dma = ring.dma_start(**kw)
                prev = last_on.get(ring_key)
                if prev is not None:
                    add_dep_helper(dma.ins, prev.ins, sync=False,
                                   reason="ring issue order")
                last_on[ring_key] = dma
                return dma

            w1_sb = const.tile([128, NSUP, 2, MLP], fp8)
            ring_dma('B', nc.scalar, out=w1_sb[:], in_=w1_d[:])
            ca_sb = const.tile([P32, 512], f32)
            nc.gpsimd.dma_start(out=ca_sb[:], in_=ca_d[:])
            cb_sb = const.tile([MLP, 2], f32)
            nc.gpsimd.dma_start(out=cb_sb[:], in_=cb_d[:])
            cwr_sb = const.tile([MLP, MLP + 2 * P32 * NT], bf16)
            nc.gpsimd.dma_start(out=cwr_sb[:], in_=cwr_d[:])

            stats_sb = statsp.tile([P32, 2], f32)

            # One 512KB transfer per super-chunk, strictly alternating
            # rings: fine granularity keeps chunk latency ~1.5us so the PE
            # never idles past the HAM re-throttle window during the ramp.
            rings = [('A', nc.sync), ('B', nc.scalar)]
            x_tiles = []
            for g in range(NSUP - 2):
                x_sb = xp.tile([128, 2, TOK], fp8, tag="x", name=f"x{g}")
                rk, ring = rings[g % 2]
                ring_dma(rk, ring, out=x_sb[:], in_=xt_d[:, g])
                x_tiles.append(x_sb)
            xl_tiles = []
            for t in range(NT):
                xl_sb = xp.tile([128, 2, 2, 512], fp8, tag="xl", name=f"xl{t}")
                rk, ring = rings[t % 2]
                ring_dma(rk, ring, out=xl_sb[:],
                         in_=xt_d[:, NSUP - 2:NSUP, :, t * 512:(t + 1) * 512])
                xl_tiles.append(xl_sb)

            # --- Layer 1: yT[t] (128, 512) += (64*W1)^T-chunk @ x-chunk,
            # DoubleRow accumulation over 16 super-chunks of K=256.
            psum_y = [ps_y.tile([128, 512], f32, tag="y", name=f"y_ps{t}")
                      for t in range(NT)]
            for g in range(NSUP - 2):
                xg = x_tiles[g]
                for t in range(NT):
                    mm = nc.tensor.matmul(
                        psum_y[t][:],
                        w1_sb[:, g],
                        xg[:, :, t * 512:(t + 1) * 512],
                        start=(g == 0), stop=False,
                        perf_mode=DR,
                    )
                    if g == 0 and t == 0 and warm_mm is not None:
                        add_dep_helper(mm.ins, warm_mm.ins, sync=False,
                                       reason="warmups before first mm")
            for t in range(NT):
                nc.tensor.matmul(psum_y[t][:], w1_sb[:, NSUP - 2],
                                 xl_tiles[t][:, 0],
                                 start=False, stop=False, perf_mode=DR)
                nc.tensor.matmul(psum_y[t][:], w1_sb[:, NSUP - 1],
                                 xl_tiles[t][:, 1],
                                 start=False, stop=True, perf_mode=DR)

            # cb/ca lane warmups (each engine observes the const lanes once
            # so steady-state instructions carry at most one new wait).
            warm_act = const.tile([MLP, 1], f32)
            nc.scalar.copy(out=warm_act[:], in_=cb_sb[:, 0:1])
            warm_dve = const.tile([P32, 1], f32)
            nc.vector.tensor_copy(out=warm_dve[:], in_=ca_sb[:, 0:1])
            warm_pe2 = ps_y2.tile([128, 512], f32, tag="y2", name="warm_pe2")
            nc.tensor.matmul(warm_pe2[0:NH, 0:NH], cwr_sb[:, 0:NH],
                             cwr_sb[:, 0:NH], start=True, stop=True)

            # --- MLP tail + head projections per token tile (bf16).
            q32_ps = ps_q.tile([P32, 512], f32, tag="q", name="q32_ps")
            v32_ps = ps_v.tile([P32, 512], f32, tag="v", name="v32_ps")
            for t in range(NT):
                y_sb = yp.tile([128, 512], bf16, tag="ysb", name=f"y_sb{t}")
                # relu on DVE (add+max) keeps ACT free for relu2/exp; the
                # 64x scale rides along and is cancelled by W2/64 in cwr.
                nc.vector.tensor_scalar(out=y_sb[:], in0=psum_y[t][:],
                                        scalar1=cb_sb[:, 0:1],
                                        scalar2=0.0, op0=OP.add, op1=OP.max)
                y2_ps = ps_y2.tile([128, 512], f32, tag="y2", name=f"y2_ps{t}")
                nc.tensor.matmul(y2_ps[:], cwr_sb[:, 0:MLP], y_sb[:],
                                 start=True, stop=True)
                y2_sb = y2p.tile([128, 512], bf16, tag="y2sb", name=f"y2_sb{t}")
                nc.scalar.activation(out=y2_sb[:], in_=y2_ps[:], func=AF.Relu,
                                     bias=cb_sb[:, 1:2], scale=1.0)
                # Head projections: the (128, 32) weight block for tile t is
                # zero outside rows 8t..8t+8, so accumulating all 4 tiles into
                # one (32, 512) bank packs q/v as (tile, head) x seq lanes.
                nc.tensor.matmul(q32_ps[:],
                                 cwr_sb[:, CQ + P32 * t:CQ + P32 * (t + 1)],
                                 y2_sb[:], start=(t == 0), stop=(t == NT - 1))
                nc.tensor.matmul(v32_ps[:],
                                 cwr_sb[:, CV + P32 * t:CV + P32 * (t + 1)],
                                 y2_sb[:], start=(t == 0), stop=(t == NT - 1))

            # --- Softmax stats over the packed (32, 512) lanes.
            # ca already contains ramp + mask - K, so l' = q + ca is the
            # shifted logit; no max-reduce needed on device.
            l_sb = smallp.tile([P32, 512], f32, tag="l", name="l_sb")
            nc.vector.tensor_add(out=l_sb[:], in0=q32_ps[:],
                                 in1=ca_sb[:])
            e_sb = smallp.tile([P32, 512], f32, tag="e", name="e_sb")
            # e = exp(l'); stats[:, 0] = Z = sum e
            nc.scalar.activation(out=e_sb[:], in_=l_sb[:], func=AF.Exp,
                                 bias=0.0, scale=1.0,
                                 accum_out=stats_sb[:, 0:1])
            ev_sb = smallp.tile([P32, 512], f32, tag="ev", name="ev_sb")
            nc.vector.tensor_mul(out=ev_sb[:], in0=e_sb[:], in1=v32_ps[:])
            # stats[:, 1] = W' = sum e*v_raw   (bv folded in on host)
            nc.vector.tensor_reduce(out=stats_sb[:, 1:2], in_=ev_sb[:],
                                    axis=AX.X, op=OP.add)

            ring_dma('A', nc.sync, out=st_d[:], in_=stats_sb[:])

    nc.finalize()
    return nc


def get_nc():
    if "nc" not in _cache:
        _cache["nc"] = _build_nc()
    return _cache["nc"]


def make_core_inputs(x, mask, W1, b1, W2, b2, Wq, Wv, bv, pos_w, bias):
    """Host-side shard + transpose + fp8 quantization.

    Returns (in_maps, K) where K[c, t, h] is the logit shift baked into
    core c's `ca` (the host-known max of ramp+mask over the shard)."""
    import ml_dtypes
    fp8 = ml_dtypes.float8_e4m3
    bf16 = ml_dtypes.bfloat16

    # w1s[p, k, m] = 64*W1[m, k*128+p], fp8
    w1s = np.ascontiguousarray(
        (W1 * W1_SCALE).reshape(MLP, KCH, 128).transpose(2, 1, 0)
    ).astype(fp8).reshape(128, NSUP, 2, MLP)

    cwr = np.zeros((MLP, MLP + 2 * P32 * NT), dtype=np.float32)
    cwr[:, 0:MLP] = W2.T / W1_SCALE
    # zero-padded per-tile head blocks: block t covers psum rows 8t..8t+8
    for t in range(NT):
        cwr[:, MLP + P32 * t + NH * t:MLP + P32 * t + NH * (t + 1)] = Wq.T
        base_v = MLP + P32 * NT
        cwr[:, base_v + P32 * t + NH * t:base_v + P32 * t + NH * (t + 1)] = \
            Wv.T
    cwr = cwr.astype(bf16)
    cb = np.stack([b1 * W1_SCALE, b2], axis=1).astype(np.float32)  # (MLP, 2)
    pos = np.arange(S, dtype=np.float32)
    maskadd = np.where(mask == 0, np.float32(-1e9), np.float32(0.0))  # (B,S)

    in_maps = []
    K_all = np.empty((NCORES, NT, NH), dtype=np.float64)
    for c in range(NCORES):
        sl = slice(c * S_SHARD, (c + 1) * S_SHARD)
        # xt[p, k, n] = x^T[k*128+p, n] for this shard's 2048 tokens
        xt = np.ascontiguousarray(
            x[:, sl, :].astype(fp8).transpose(2, 0, 1).reshape(H, TOK)
            .reshape(KCH, 128, TOK).transpose(1, 0, 2)
        ).reshape(128, NSUP, 2, TOK)
        add_ths = (pos_w.astype(np.float64)[None, :, None]
                   * pos[sl].astype(np.float64)[None, None, :]
                   + maskadd[:, None, sl])           # (B=NT, NH, 512)
        # K: host-known shift; floor keeps exp() sane when fully masked
        K = np.maximum(add_ths.max(axis=2), -60.0)   # (NT, NH)
        K_all[c] = K
        ca = (add_ths - K[:, :, None]).astype(np.float32).reshape(P32, 512)
        in_maps.append({"xt": xt, "w1s": w1s, "cwr": cwr, "cb": cb,
                        "ca": ca})
    return in_maps, K_all


def merge_stats(stats_all, K_all, bv, bias):
    """stats_all: (NCORES, 32, 2) = [Z, W'] per (batch tile, head) lane
    under shift K_all[c, t, h]; W = W' + bv*Z -> (B, 1) output."""
    st = np.asarray(stats_all, dtype=np.float64).reshape(NCORES, NT, NH, 2)
    m = K_all                # (C, B, NH): logits were shifted by -K
    Z = st[..., 0]
    W = st[..., 1] + bv.astype(np.float64)[None, None, :] * Z
    M = m.max(axis=0)        # (B, NH)
    alpha = np.exp(m - M[None])
    Zg = (alpha * Z).sum(axis=0)
    Wg = (alpha * W).sum(axis=0)
    out = (Wg / Zg).sum(axis=1)          # (B,)
    return (out[:, None] + np.float64(bias.reshape(1)[0])).astype(np.float32)


def kernel(x, mask, W1, b1, W2, b2, Wq, Wv, bv, pos_w, bias, _trace=False):
    from concourse.bass_utils import run_bass_kernel_spmd

    x = np.asarray(x, dtype=np.float32)
    args = [np.asarray(a) for a in (W1, b1, W2, b2, Wq, Wv, bv, pos_w, bias)]
    in_maps, K_all = make_core_inputs(x, np.asarray(mask), *args)
    nc = get_nc()
    res = run_bass_kernel_spmd(nc, in_maps, core_ids=list(range(NCORES)),
                               trace=_trace)
    stats_all = np.stack([r["stats"] for r in res.results])  # (C, 32, 2)
    out = merge_stats(stats_all, K_all, args[6], args[8])
    if _trace:
        kernel.last_result = res
    return out
